# revision 12
# baseline (speedup 1.0000x reference)
"""ArgMaxTop Trainium2 kernel.

Math: out[b] = argmax_c sum_s x[b,s,c] * [x[b,s,c] >= t(b,s)] where t is the
8th-largest value of row (b,s). This equals the reference's
scatter-top8/mean/argmax pipeline for inputs without exact float ties (the
mean-over-s divides every class by the same S, so argmax is unchanged; absent
classes sum to 0 < any winner).

Sharding: batch b -> core b (8 batches, 8 cores), no collectives.

Per-core dataflow, per 128-row s-tile (16 tiles):
  - DMA x chunks [128, 8000] f32 to SBUF (pool of 5 bufs for overlap)
  - DVE `max` (top-8 per partition) per chunk -> concat [128,32] -> max -> t
  - ACT: r = Relu(x - t)   (bias = -t per partition)
  - DVE: m = (x >= t) * t  (fused tensor_scalar, runs in 2x_2p fp32 mode)
  - PE (float32r): psum[64,500] += onehot_j^T @ r_win + onehot_j^T @ m_win for
    each of 64 500-wide class windows; onehot_j is a shifted window of a
    constant [128,128] matrix with ones in column 64, routing window j to psum
    row j. PSUM accumulation over both streams and all 16 tiles yields
    sums[c] = sum relu(x-t) + sum t*(x>=t) = sum x*[x>=t].
  - drain psum -> SBUF -> DRAM out [64, 500]; host argmaxes the 32000 sums.

GpSimd/Pool is NOT used for elementwise work (it is a DSP, ~60x slower than
DVE line rate on TRN2).
"""

import sys

if "/opt/trn_rl_repo" not in sys.path:
    sys.path.insert(0, "/opt/trn_rl_repo")

import numpy as np

B, S, C = 8, 2048, 32000
TOP_K = 8
P = 128          # partitions per s-tile
XCH = 8000       # x chunk width (DVE max input free size <= 16384)
RCH = 1000       # relu (ACT) chunk width
MCH = 2000       # mask (DVE) chunk width
CCH = 500        # matmul moving window / psum columns
NROWS = C // CCH  # 64 psum rows

USE_F32R = True  # float32r matmuls: 1 cycle/row vs 4 for plain fp32

_CACHE = {}


def _build_graph(s_len=S, x_bufs=5, use_f32r=None):
    from concourse import bacc, tile, mybir

    if use_f32r is None:
        use_f32r = USE_F32R
    f32 = mybir.dt.float32
    f32r = mybir.dt.float32r
    mmdt = f32r if use_f32r else f32
    Alu = mybir.AluOpType
    Act = mybir.ActivationFunctionType

    nc = bacc.Bacc("TRN2", target_bir_lowering=False, debug=False)
    x = nc.dram_tensor("x", [s_len, C], f32, kind="ExternalInput").ap()
    zc = nc.dram_tensor("zcols", [P, P], f32, kind="ExternalInput").ap()
    out = nc.dram_tensor("out", [NROWS, CCH], f32, kind="ExternalOutput").ap()

    ntiles = s_len // P
    nxch = C // XCH
    n_mm = ntiles * (C // CCH) * 2
    mm_i = 0

    with tile.TileContext(nc) as tc:
        with (
            tc.tile_pool(name="consts", bufs=1) as consts,
            tc.tile_pool(name="xp", bufs=x_bufs) as xp,
            tc.tile_pool(name="tp", bufs=3) as tp,
            tc.tile_pool(name="rp", bufs=3) as rp,
            tc.tile_pool(name="mp", bufs=3) as mp,
            tc.tile_pool(name="sump", bufs=1) as sump,
            tc.tile_pool(name="ps", bufs=1, space="PSUM") as ps,
        ):
            zt = consts.tile([P, P], f32, name="zt")
            nc.sync.dma_start(out=zt, in_=zc)
            if use_f32r:
                ztm = consts.tile([P, P], mmdt, name="ztm")
                nc.vector.tensor_copy(ztm, zt)
            else:
                ztm = zt

            acc = ps.tile([NROWS, CCH], f32, name="acc")

            for it in range(ntiles):
                xch = []
                for j in range(nxch):
                    xt = xp.tile([P, XCH], f32, name="xch", tag="xch")
                    nc.sync.dma_start(
                        out=xt,
                        in_=x[it * P : (it + 1) * P, j * XCH : (j + 1) * XCH],
                    )
                    xch.append(xt)

                top = tp.tile([P, 8 * nxch], f32, name="top", tag="top")
                for j in range(nxch):
                    nc.vector.max(out=top[:, 8 * j : 8 * (j + 1)], in_=xch[j])
                top8 = tp.tile([P, 8], f32, name="top8", tag="top8")
                nc.vector.max(out=top8, in_=top)
                t_ap = top8[:, 7:8]
                tneg = tp.tile([P, 1], f32, name="tneg", tag="tneg")
                nc.scalar.activation(
                    out=tneg, in_=t_ap, func=Act.Copy, scale=-1.0
                )

                for j in range(nxch):
                    for ms in range(XCH // MCH):
                        xs = xch[j][:, ms * MCH : (ms + 1) * MCH]
                        m = mp.tile([P, MCH], mmdt, name="m", tag="m")
                        nc.vector.tensor_scalar(
                            m, xs, t_ap, t_ap, Alu.is_ge, Alu.mult
                        )
                        rtiles = []
                        for rs in range(MCH // RCH):
                            off = ms * MCH + rs * RCH
                            r = rp.tile([P, RCH], mmdt, name="r", tag="r")
                            nc.scalar.activation(
                                out=r,
                                in_=xch[j][:, off : off + RCH],
                                func=Act.Relu,
                                bias=tneg,
                                scale=1.0,
                            )
                            rtiles.append(r)
                        for w in range(MCH // CCH):
                            cg = (j * XCH + ms * MCH + w * CCH) // CCH
                            stat = ztm[:, 64 - cg : 128 - cg]
                            off = w * CCH
                            rsrc = rtiles[off // RCH][
                                :, off % RCH : off % RCH + CCH
                            ]
                            msrc = m[:, off : off + CCH]
                            nc.tensor.matmul(
                                acc,
                                stat,
                                rsrc,
                                start=(mm_i == 0),
                                stop=(mm_i == n_mm - 1),
                            )
                            mm_i += 1
                            nc.tensor.matmul(
                                acc,
                                stat,
                                msrc,
                                start=False,
                                stop=(mm_i == n_mm - 1),
                            )
                            mm_i += 1

            sums = sump.tile([NROWS, CCH], f32, name="sums")
            nc.scalar.activation(out=sums, in_=acc, func=Act.Copy)
            nc.sync.dma_start(out=out, in_=sums)

    nc.compile()
    return nc


def _zcols():
    zc = np.zeros((P, P), dtype=np.float32)
    zc[:, 64] = 1.0
    return zc


def kernel(**inputs):
    from concourse import bass_utils

    x = np.asarray(inputs["inputs"], dtype=np.float32)
    assert x.shape == (B, S, C), x.shape

    if "nc" not in _CACHE:
        _CACHE["nc"] = _build_graph()
    nc = _CACHE["nc"]

    zc = _zcols()
    in_maps = [
        {"x": np.ascontiguousarray(x[b]), "zcols": zc} for b in range(B)
    ]
    res = bass_utils.run_bass_kernel_spmd(nc, in_maps, core_ids=list(range(B)))

    out = np.empty((B,), dtype=np.int32)
    for b in range(B):
        sums = np.asarray(res.results[b]["out"], dtype=np.float32).reshape(-1)
        out[b] = np.argmax(sums)
    return out
